# revision 6
# baseline (speedup 1.0000x reference)
"""Trainium2 Bass kernel for nn_STSourceModule (segment_reduce).

Math: source_ids x are binary {0,1}, so the masked softmax over sites
collapses to a closed form.  With g[n] = exp(fire_bias[n]),
A0[h] = exp(attn_b[h]), A1[h] = exp(attn_b[h] + attn_w[h]):

  Z[s,h,c]   = A0[h]*(S0[c] - T1[s,c]) + A1[h]*T1[s,c]
  r[s,c,h]   = A1[h]*T1[s,c] / Z[s,h,c]          (fraction of prob mass on x=1 sites)
  out[s,c,:] = mask[c]*base + sum_h r[s,c,h]*Wh[h,:]

where S0[c] = sum_{n in c} g[n], T1[s,c] = sum_{n in c} x[s,n]*g[n],
base = ffn_b + ffn_w@val_b, Wh[h] = ffn_w[:,32h:32h+32]@val_w[32h:32h+32].

Sharding: data-parallel over batch B=8, one batch element per core.
"""

import sys

for _p in ("/opt/trn_rl_repo",):
    if _p not in sys.path:
        sys.path.insert(0, _p)

from contextlib import ExitStack

import numpy as np

import concourse.bass as bass
import concourse.tile as tile
from concourse import bacc, mybir
from concourse.bass_utils import run_bass_kernel_spmd
from concourse.masks import make_identity

F32 = mybir.dt.float32
AF = mybir.ActivationFunctionType
ALU = mybir.AluOpType

MAX_SP, MAX_TP = 180.0, 365.0
B, S, N, C = 8, 256, 128, 64
NH, HID, FH = 4, 256, 32

TRACE = False           # set True (e.g. from test.py) to neuron-profile
LAST_RESULT = None      # BassKernelResults of the last run


def _build_program(csp, ctp, a0, a1):
    """Trace the per-core Bass program. csp/ctp and the per-head exp()'d
    attention scalars are baked as immediates."""
    nc = bacc.Bacc()

    x_d = nc.declare_dram_parameter("x", [S, N], F32, isOutput=False)
    lab_d = nc.declare_dram_parameter("lab", [N, 1], F32, isOutput=False)
    dsp_d = nc.declare_dram_parameter("dsp", [N, 1], F32, isOutput=False)
    dtp_d = nc.declare_dram_parameter("dtp", [N, 1], F32, isOutput=False)
    w1s_d = nc.declare_dram_parameter("w1s", [1, FH], F32, isOutput=False)
    w2s_d = nc.declare_dram_parameter("w2s", [1, FH], F32, isOutput=False)
    w1t_d = nc.declare_dram_parameter("w1t", [1, FH], F32, isOutput=False)
    w2t_d = nc.declare_dram_parameter("w2t", [1, FH], F32, isOutput=False)
    iota_d = nc.declare_dram_parameter("iotac", [1, C], F32, isOutput=False)
    waug_d = nc.declare_dram_parameter("waug", [5, HID], F32, isOutput=False)
    out_d = nc.declare_dram_parameter("out", [S, C, HID], F32, isOutput=True)

    with tile.TileContext(nc) as tc, ExitStack() as ctx:
        consts = ctx.enter_context(tc.tile_pool(name="consts", bufs=1))
        work = ctx.enter_context(tc.tile_pool(name="work", bufs=2))
        tpsum = ctx.enter_context(tc.tile_pool(name="tpsum", bufs=2, space="PSUM"))
        psum = ctx.enter_context(tc.tile_pool(name="psum", bufs=1, space="PSUM"))
        opsum = ctx.enter_context(tc.tile_pool(name="opsum", bufs=2, space="PSUM"))
        outp = ctx.enter_context(tc.tile_pool(name="outp", bufs=3))

        # ---- constants -------------------------------------------------
        ident = consts.tile([128, 128], F32)
        make_identity(nc, ident)

        iot = consts.tile([128, C], F32)
        nc.sync.dma_start(out=iot, in_=iota_d[:, :].to_broadcast([128, C]))

        w1cat = consts.tile([128, 2 * FH], F32)   # [silu-in weights sp | tp]
        nc.sync.dma_start(out=w1cat[:, 0:FH], in_=w1s_d[:, :].to_broadcast([128, FH]))
        nc.sync.dma_start(out=w1cat[:, FH:], in_=w1t_d[:, :].to_broadcast([128, FH]))
        w2cat = consts.tile([128, 2 * FH], F32)
        nc.sync.dma_start(out=w2cat[:, 0:FH], in_=w2s_d[:, :].to_broadcast([128, FH]))
        nc.sync.dma_start(out=w2cat[:, FH:], in_=w2t_d[:, :].to_broadcast([128, FH]))

        waug = consts.tile([5, HID], F32)
        nc.sync.dma_start(out=waug, in_=waug_d[:, :])

        ones = consts.tile([128, 1], F32)
        nc.vector.memset(ones, 1.0)

        labs = consts.tile([128, 1], F32)
        nc.sync.dma_start(out=labs, in_=lab_d[:, :])
        dsp = consts.tile([128, 1], F32)
        nc.sync.dma_start(out=dsp, in_=dsp_d[:, :])
        dtp = consts.tile([128, 1], F32)
        nc.sync.dma_start(out=dtp, in_=dtp_d[:, :])

        x0 = work.tile([128, N], F32)
        nc.sync.dma_start(out=x0, in_=x_d[0:128, :])
        x1 = work.tile([128, N], F32)
        nc.sync.dma_start(out=x1, in_=x_d[128:256, :])

        # ---- FIRE bias -> g = exp(bias) (per site n, on partitions) ----
        dls = work.tile([128, 1], F32)
        nc.scalar.activation(out=dls, in_=dsp, func=AF.Ln, bias=1.0, scale=csp)
        dlt = work.tile([128, 1], F32)
        nc.scalar.activation(out=dlt, in_=dtp, func=AF.Ln, bias=1.0, scale=ctp)
        hcat = work.tile([128, 2 * FH], F32)
        nc.vector.tensor_scalar_mul(out=hcat[:, 0:FH], in0=w1cat[:, 0:FH], scalar1=dls)
        nc.vector.tensor_scalar_mul(out=hcat[:, FH:], in0=w1cat[:, FH:], scalar1=dlt)
        nc.scalar.activation(out=hcat, in_=hcat, func=AF.Silu)
        nc.vector.tensor_mul(out=hcat, in0=hcat, in1=w2cat)
        bsum = work.tile([128, 1], F32)
        nc.vector.reduce_sum(out=bsum, in_=hcat, axis=mybir.AxisListType.X)
        g = work.tile([128, 1], F32)
        nc.scalar.activation(out=g, in_=bsum, func=AF.Exp)

        # ---- membership * g: mg[n,c] = (lab[n]==c) * g[n] --------------
        mg = work.tile([128, C], F32)
        nc.vector.tensor_scalar(
            out=mg, in0=iot, scalar1=labs, scalar2=g,
            op0=ALU.is_equal, op1=ALU.mult,
        )

        # ---- xT via PE transpose ---------------------------------------
        # A transpose matmul lowers to a lone LDWEIGHTS, which has a single
        # sync-wait slot.  Warm PE's view of the Pool semaphore (identity
        # generation) with a dummy transpose so the real transposes only
        # need to wait on their input DMA.
        ptd = tpsum.tile([128, 128], F32, tag="pt")
        nc.tensor.transpose(ptd, ident, ident)
        xT = work.tile([128, S], F32)
        for i, xi in enumerate((x0, x1)):
            pt = tpsum.tile([128, 128], F32, tag="pt")
            nc.tensor.transpose(pt, xi, ident)
            nc.scalar.copy(out=xT[:, i * 128:(i + 1) * 128], in_=pt)

        # ---- T1t[c,s] and S0[c] ----------------------------------------
        t1_ps = psum.tile([64, S], F32)
        nc.tensor.matmul(t1_ps, lhsT=mg, rhs=xT, start=True, stop=True)
        s0_ps = psum.tile([64, 1], F32)
        nc.tensor.matmul(s0_ps, lhsT=mg, rhs=ones, start=True, stop=True)

        t1 = work.tile([64, S], F32)
        nc.vector.tensor_copy(out=t1, in_=t1_ps)
        s0 = work.tile([64, 1], F32)
        nc.scalar.copy(out=s0, in_=s0_ps)

        mask = work.tile([64, 1], F32)
        nc.vector.tensor_scalar(out=mask, in0=s0, scalar1=0.0, scalar2=None,
                                op0=ALU.is_gt)
        maskc = work.tile([64, 1], F32)
        nc.vector.tensor_scalar(out=maskc, in0=s0, scalar1=0.0, scalar2=None,
                                op0=ALU.is_le)

        # ---- r planes: rall[c, j, s], j=0 mask, j=1..4 heads -----------
        rall = work.tile([64, 5, S], F32)
        # mask plane: 0*t1 + mask  (broadcast mask along s)
        nc.vector.tensor_scalar(out=rall[:, 0, :], in0=t1, scalar1=0.0,
                                scalar2=mask, op0=ALU.mult, op1=ALU.add)
        for h in range(NH):
            a0h, a1h = float(a0[h]), float(a1[h])
            sam = work.tile([64, 1], F32)
            # A0*S0 + (1-mask): empty clusters get denominator 1
            nc.vector.tensor_scalar(out=sam, in0=s0, scalar1=a0h, scalar2=maskc,
                                    op0=ALU.mult, op1=ALU.add)
            den = work.tile([64, S], F32)
            nc.vector.tensor_scalar(out=den, in0=t1, scalar1=a1h - a0h,
                                    scalar2=sam, op0=ALU.mult, op1=ALU.add)
            rinv = work.tile([64, S], F32)
            nc.vector.reciprocal(out=rinv, in_=den)
            nc.vector.scalar_tensor_tensor(
                out=rall[:, 1 + h, :], in0=t1, scalar=a1h, in1=rinv,
                op0=ALU.mult, op1=ALU.mult,
            )

        # ---- flatten to Rt[j, c, s] (j on partitions) ------------------
        rt = consts.tile([5, C, S], F32)
        for j in range(5):
            nc.sync.dma_start(out=rt[j:j + 1, :, :], in_=rall[:, j, :])

        # ---- expansion: out[s,c,:] = Rt[:,c,s]^T @ waug ----------------
        CQ = 4          # c values per PSUM tile (2 banks)
        it = 0
        for sh in range(2):
            srange = slice(sh * 128, (sh + 1) * 128)
            for cq in range(C // CQ):
                ps = opsum.tile([128, CQ, HID], F32, tag="ops")
                for cc in range(CQ):
                    nc.tensor.matmul(
                        ps[:, cc, :],
                        lhsT=rt[:, cq * CQ + cc, srange],
                        rhs=waug,
                        start=True, stop=True,
                    )
                st = outp.tile([128, CQ, HID], F32, tag="st")
                if it % 2 == 0:
                    nc.vector.tensor_copy(out=st, in_=ps)
                else:
                    nc.scalar.copy(out=st, in_=ps)
                nc.sync.dma_start(
                    out=out_d[srange, cq * CQ:(cq + 1) * CQ, :], in_=st
                )
                it += 1

    nc.finalize()
    return nc


_CACHE = {}


def _program(csp, ctp, a0, a1):
    key = (csp, ctp, tuple(a0), tuple(a1))
    if key not in _CACHE:
        _CACHE[key] = _build_program(csp, ctp, a0, a1)
    return _CACHE[key]


def kernel(source_ids, source_cluster_labels, in_cluster_spatial_dist,
           in_cluster_temporal_dist, num_clusters,
           c_sp, sp_w1, sp_w2, c_tp, tp_w1, tp_w2,
           attn_w, attn_b, val_w, val_b, ffn_w, ffn_b):
    global LAST_RESULT

    x = np.ascontiguousarray(np.asarray(source_ids), dtype=np.float32)
    lab = np.asarray(source_cluster_labels).astype(np.float32)
    dsp = np.asarray(in_cluster_spatial_dist).astype(np.float32)
    dtp = np.asarray(in_cluster_temporal_dist).astype(np.float32)
    assert int(np.asarray(num_clusters)) == C

    csp = float(max(float(np.asarray(c_sp)), 0.0))
    ctp = float(max(float(np.asarray(c_tp)), 0.0))
    lsp = float(np.log(csp * MAX_SP + 1.0))
    ltp = float(np.log(ctp * MAX_TP + 1.0))

    sp_w1 = np.asarray(sp_w1, dtype=np.float32)   # (FH,1)
    sp_w2 = np.asarray(sp_w2, dtype=np.float32)   # (1,FH)
    tp_w1 = np.asarray(tp_w1, dtype=np.float32)
    tp_w2 = np.asarray(tp_w2, dtype=np.float32)
    w1s = np.ascontiguousarray((sp_w1[:, 0] / lsp)[None, :], dtype=np.float32)
    w2s = np.ascontiguousarray(sp_w2, dtype=np.float32)
    w1t = np.ascontiguousarray((tp_w1[:, 0] / ltp)[None, :], dtype=np.float32)
    w2t = np.ascontiguousarray(tp_w2, dtype=np.float32)

    attn_w = np.asarray(attn_w, dtype=np.float64)  # (4,)
    attn_b = np.asarray(attn_b, dtype=np.float64)
    a0 = np.exp(attn_b)
    a1 = np.exp(attn_b + attn_w)

    val_w = np.asarray(val_w, dtype=np.float64)    # (128,)
    val_b = np.asarray(val_b, dtype=np.float64)
    ffn_w = np.asarray(ffn_w, dtype=np.float64)    # (256,128)
    ffn_b = np.asarray(ffn_b, dtype=np.float64)
    waug = np.zeros((5, HID), dtype=np.float64)
    waug[0] = ffn_b + ffn_w @ val_b
    for h in range(NH):
        blk = slice(h * 32, (h + 1) * 32)
        waug[1 + h] = ffn_w[:, blk] @ val_w[blk]
    waug = np.ascontiguousarray(waug, dtype=np.float32)

    iotac = np.arange(C, dtype=np.float32)[None, :]

    nc = _program(csp, ctp, tuple(a0.tolist()), tuple(a1.tolist()))

    in_maps = []
    for b in range(B):
        in_maps.append({
            "x": x[b],
            "lab": np.ascontiguousarray(lab[b][:, None]),
            "dsp": np.ascontiguousarray(dsp[b][:, None]),
            "dtp": np.ascontiguousarray(dtp[b][:, None]),
            "w1s": w1s, "w2s": w2s, "w1t": w1t, "w2t": w2t,
            "iotac": iotac, "waug": waug,
        })

    res = run_bass_kernel_spmd(nc, in_maps, core_ids=list(range(B)),
                               trace=TRACE)
    LAST_RESULT = res
    out = np.stack([res.results[b]["out"] for b in range(B)], axis=0)
    return out
